# revision 1
# baseline (speedup 1.0000x reference)
# Multi-head causal attention (B=1, T=4096, D=1024, H=16) on 8 TRN2 NeuronCores.
#
# Sharding: tensor-parallel over heads. Core n computes head channels
# [128n, 128n+128) (= heads 2n, 2n+1), runs the full causal attention for its
# two heads, and produces a full-shape partial output
#   y_n = attn_out[:, ch_n] @ Wo[:, ch_n].T        (4096, 1024)
# The host sums the 8 partials (row-sharded Wo contraction) — no collectives.
#
# Device-side layout (per core):
#   xT   [1024, 4096]  x transposed, bf16 (host-prepped) — contraction on partitions
#   QT/KT [128, 4096]  head channels on partitions (h0: 0-63, h1: 64-127), bf16
#   scoresT[j, i]      keys on partitions, queries on free dim; the softmax sum
#                      over keys rides the PV matmul via a ones-column appended
#                      to V (V' = [V | 1], M=65): psum row 64 = rowsum.
#   exp on ScalarE directly PSUM->SBUF (bf16 out) with 1/sqrt(dk) folded into
#   the activation scale; psum accumulation is always fp32.
#   Causal: only key tiles with j <= i are computed; diagonal 128x128 blocks
#   are masked by a 0/1 upper-triangular multiply after exp.
#
# Measured on 8 axon TRN2 cores: ~333 us HW exec (slowest core),
# rel L2 error vs fp32 reference ~5.9e-3.

import os
import sys

for _p in ("/opt/trn_rl_repo", "/root/.axon_site/_ro/trn_rl_repo"):
    if os.path.isdir(_p) and _p not in sys.path:
        sys.path.insert(0, _p)

import ml_dtypes
import numpy as np

def _ensure_axon_ntff_hook():
    """The agent image's antenv package lacks axon_hooks, which makes
    run_bass_kernel_spmd(trace=True) crash at import under axon. Provide the
    module and register the boot hook so NTFF profiling works."""
    import types

    try:
        import antenv.axon_hooks  # noqa: F401
        return
    except ImportError:
        pass
    try:
        import antenv
    except ImportError:
        return
    mod = types.ModuleType("antenv.axon_hooks")
    mod._hook = None
    mod.set_axon_ntff_profile_hook = lambda h: setattr(mod, "_hook", h)
    mod.get_axon_ntff_profile_hook = lambda: mod._hook
    sys.modules["antenv.axon_hooks"] = mod
    antenv.axon_hooks = mod
    try:
        from trn_agent_boot.trn_boot import _ntff_profile_via_ctypes

        so = "/opt/axon/libaxon_pjrt.so"
        if os.path.exists(so):
            mod._hook = _ntff_profile_via_ctypes(so)
    except Exception:
        pass


_ensure_axon_ntff_hook()

import concourse.bass as bass
import concourse.tile as tile
from concourse import bacc
from concourse import mybir
from concourse.bass_utils import run_bass_kernel_spmd

F32 = mybir.dt.float32
BF16 = mybir.dt.bfloat16
EXP = mybir.ActivationFunctionType.Exp
NPBF = ml_dtypes.bfloat16

D = 1024          # d_model
DK = 64           # head dim
CPC = 128         # channels per core (2 heads)
ICH = 512         # query-chunk size
IH = 512          # i-half width (matmul N / psum bank limit)
JT = 128          # key-tile size

# If True: assemble outT[128, i] with DVE copies that shift partition base
# (psum parts 0-63 -> sbuf parts 64-127) and do the output projection as one
# K=128 matmul.  If False: keep the two heads in separate base-0 tiles and do
# the projection as two K=64 accumulating matmuls.
XPART_COPY = True

_NC_CACHE = {}


def build(T, xpart_copy=XPART_COPY, dbg=False):
    """Build the per-core Bass program for sequence length T."""
    nc = bacc.Bacc(None, target_bir_lowering=False, debug=False)
    ich = min(ICH, T)
    nih = ich // IH
    nch = T // ich
    if dbg:
        qt_dbg = nc.dram_tensor("qt_dbg", [128, 2, T], BF16, kind="ExternalOutput")
        vp_dbg = nc.dram_tensor(
            "vp_dbg", [128, T // JT, 2 * (DK + 1)], BF16, kind="ExternalOutput"
        )
        ex_dbg = nc.dram_tensor(
            "ex_dbg", [2, T // JT, JT, ICH], BF16, kind="ExternalOutput"
        )
        outtraw_dbg = nc.dram_tensor(
            "outtraw_dbg", [128, T], BF16, kind="ExternalOutput"
        )

    xT_d = nc.dram_tensor("xT", [D, T], BF16, kind="ExternalInput")
    wqT_d = nc.dram_tensor("wqT", [D, CPC], BF16, kind="ExternalInput")
    wkT_d = nc.dram_tensor("wkT", [D, CPC], BF16, kind="ExternalInput")
    wvT_d = nc.dram_tensor("wvT", [D, CPC], BF16, kind="ExternalInput")
    woT_d = nc.dram_tensor("woT", [CPC, D], BF16, kind="ExternalInput")
    tri_d = nc.dram_tensor("tri", [JT, JT], BF16, kind="ExternalInput")
    ident_d = nc.dram_tensor("ident", [128, 128], BF16, kind="ExternalInput")
    y_d = nc.dram_tensor("y", [T, D], F32, kind="ExternalOutput")
    rs_scratch = nc.dram_tensor("rs_scratch", [nch, 2, ich], F32)
    rs2_scratch = nc.dram_tensor("rs2_scratch", [nch, 2 * ich], F32)

    with tile.TileContext(nc) as tc:
        with (
            tc.tile_pool(name="const", bufs=1) as const,
            tc.tile_pool(name="xtp", bufs=3) as xtp,
            tc.tile_pool(name="vtp", bufs=3) as vtp,
            tc.tile_pool(name="expp", bufs=6) as expp,
            tc.tile_pool(name="outp", bufs=3) as outp,
            tc.tile_pool(name="yp", bufs=4) as yp,
            tc.tile_pool(name="scp", bufs=1, space="PSUM") as scp,
            tc.tile_pool(name="pvp", bufs=2, space="PSUM") as pvp,
        ):
            # ---- constants / persistent state ----
            wq_sb = const.tile([128, D // 128, 128], BF16)
            wk_sb = const.tile([128, D // 128, 128], BF16)
            wv_sb = const.tile([128, D // 128, 128], BF16)
            for w_sb, w_d in ((wq_sb, wqT_d), (wk_sb, wkT_d), (wv_sb, wvT_d)):
                nc.sync.dma_start(
                    out=w_sb, in_=w_d.rearrange("(t p) c -> p t c", p=128)
                )
            wo_sb = const.tile([128, D], BF16)
            nc.sync.dma_start(out=wo_sb, in_=woT_d[:, :])
            tri_sb = const.tile([JT, JT], BF16)
            nc.sync.dma_start(out=tri_sb, in_=tri_d[:, :])
            id_sb = const.tile([128, 128], BF16)
            nc.sync.dma_start(out=id_sb, in_=ident_d[:, :])

            qt_sb = const.tile([128, 2, T], BF16)  # [:,0,:]=QT, [:,1,:]=KT
            # V' = [V_h | 1] per head: [j, jt, 2*65]
            vp_sb = const.tile([128, T // JT, 2 * (DK + 1)], BF16)
            ones_view = vp_sb.rearrange("p t (h c) -> p t h c", h=2)[
                :, :, :, DK : DK + 1
            ]
            nc.vector.memset(ones_view, 1.0)

            xT_v = xT_d.rearrange("(t p) i -> p t i", p=128)

            def emit_proj(c):
                # ---- load x^T chunk ----
                xt_ch = xtp.tile([128, D // 128, ich], BF16, tag="xt", name="xt_ch")
                nc.sync.dma_start(
                    out=xt_ch, in_=xT_v[:, :, c * ich : (c + 1) * ich]
                )
                for ihx in range(nih):
                    i0 = c * ich + ihx * IH
                    xs = slice(ihx * IH, (ihx + 1) * IH)
                    # ---- Q/K projections (into one 2-bank psum tile) ----
                    qk_ps = scp.tile([128, 2, IH], F32, tag="sch0", name="qk_ps")
                    for qk, w_sb in ((0, wq_sb), (1, wk_sb)):
                        for t in range(D // 128):
                            nc.tensor.matmul(
                                out=qk_ps[:, qk, :],
                                lhsT=w_sb[:, t, :],
                                rhs=xt_ch[:, t, xs],
                                start=(t == 0),
                                stop=(t == D // 128 - 1),
                            )
                    nc.vector.tensor_copy(
                        out=qt_sb[:, :, i0 : i0 + IH], in_=qk_ps
                    )
                    # ---- V projection (transposed), PE-transpose to natural ----
                    vt_ps = scp.tile([128, 2, IH], F32, tag="sch1", name="vt_ps")
                    for t in range(D // 128):
                        nc.tensor.matmul(
                            out=vt_ps[:, 0, :],
                            lhsT=wv_sb[:, t, :],
                            rhs=xt_ch[:, t, xs],
                            start=(t == 0),
                            stop=(t == D // 128 - 1),
                        )
                    vt_sb = vtp.tile([128, IH], BF16, tag="vt", name="vt_sb")
                    nc.vector.tensor_copy(out=vt_sb, in_=vt_ps[:, 0, :])
                    vn_ps = scp.tile(
                        [128, 2 * IH // 128, 128], BF16, tag="sch0", name="vn_ps"
                    )
                    for sdx in range(IH // 128):
                        nc.tensor.transpose(
                            out=vn_ps[:, sdx, :],
                            in_=vt_sb[:, sdx * 128 : (sdx + 1) * 128],
                            identity=id_sb,
                        )
                    jt0 = i0 // JT
                    nc.vector.tensor_copy(
                        out=vp_sb.rearrange("p t (h c) -> p t h c", h=2)[
                            :, jt0 : jt0 + IH // 128, :, 0:DK
                        ],
                        in_=vn_ps[:, 0 : IH // 128, :].rearrange(
                            "p s (h c) -> p s h c", h=2
                        ),
                    )

            def emit_attn(c):
                pv = [
                    pvp.tile([128, ich], F32, tag="pv0", name="pv0"),
                    pvp.tile([128, ich], F32, tag="pv1", name="pv1"),
                ]
                for ihx in range(nih):
                    i0 = c * ich + ihx * IH
                    njt = (i0 + IH) // JT
                    for p in range(njt // 2):
                        for h in range(2):
                            hp = slice(h * DK, (h + 1) * DK)
                            sc = scp.tile(
                                [128, 2, IH], F32, tag=f"sch{h}", name="sc"
                            )
                            for jj in range(2):
                                jt = 2 * p + jj
                                nc.tensor.matmul(
                                    out=sc[:, jj, :],
                                    lhsT=qt_sb[hp, 1, jt * JT : (jt + 1) * JT],
                                    rhs=qt_sb[hp, 0, i0 : i0 + IH],
                                    start=True,
                                    stop=True,
                                )
                            ex = expp.tile(
                                [128, 2, IH], BF16, tag="ex", name="ex"
                            )
                            nc.scalar.activation(
                                out=ex, in_=sc, func=EXP, scale=1.0 / np.sqrt(DK)
                            )
                            for jj in range(2):
                                jt = 2 * p + jj
                                off = jt * JT - i0
                                if off >= 0:  # diagonal tile: causal mask
                                    if off > 0:
                                        nc.vector.memset(ex[:, jj, 0:off], 0.0)
                                    nc.vector.tensor_mul(
                                        ex[:, jj, off : off + JT],
                                        ex[:, jj, off : off + JT],
                                        tri_sb,
                                    )
                            for jj in range(2):
                                jt = 2 * p + jj
                                nc.tensor.matmul(
                                    out=pv[h][
                                        0 : DK + 1, ihx * IH : (ihx + 1) * IH
                                    ],
                                    lhsT=vp_sb[
                                        :, jt, h * (DK + 1) : (h + 1) * (DK + 1)
                                    ],
                                    rhs=ex[:, jj, :],
                                    start=(jt == 0),
                                    stop=(jt == njt - 1),
                                )
                return pv

            def emit_tail(c, pv):
                i0 = c * ich
                # ---- softmax normalization ----
                rs = outp.tile([DK + 1, 2, ich], F32, tag="rs", name="rs")
                for h in range(2):
                    nc.vector.tensor_copy(
                        out=rs[DK : DK + 1, h, :], in_=pv[h][DK : DK + 1, :]
                    )
                nc.sync.dma_start(
                    out=rs_scratch[c : c + 1, :, :], in_=rs[DK : DK + 1, :, :]
                )
                # reciprocal spread across all 128 partitions
                rsp = outp.tile([128, 2 * ich // 128], F32, tag="rsp", name="rsp")
                nc.sync.dma_start(
                    out=rsp,
                    in_=rs_scratch[c].rearrange("h i -> (h i)").rearrange(
                        "(p f) -> p f", p=128
                    ),
                )
                nc.vector.reciprocal(out=rsp, in_=rsp)
                nc.sync.dma_start(
                    out=rs2_scratch[c].rearrange("(p f) -> p f", p=128), in_=rsp
                )
                bc = outp.tile([128, ich], F32, tag="bc", name="bc")
                for h in range(2):
                    nc.gpsimd.dma_start(
                        out=bc[h * DK : (h + 1) * DK, :],
                        in_=rs2_scratch[c].rearrange("(h i) -> h i", h=2)[
                            h : h + 1, :
                        ].to_broadcast([DK, ich]),
                    )
                outt = outp.tile([128, ich], BF16, tag="outt", name="outt")
                for h in range(2):
                    nc.vector.tensor_copy(
                        out=outt[h * DK : (h + 1) * DK, :], in_=pv[h][0:DK, :]
                    )
                if dbg:
                    nc.sync.dma_start(out=outtraw_dbg[:, i0 : i0 + ich], in_=outt)
                nc.vector.tensor_mul(outt, outt, bc)
                for sidx in range(ich // 128):
                    y_ps = [
                        pvp.tile([128, IH], F32, tag="pv0", name="ye0"),
                        pvp.tile([128, IH], F32, tag="pv1", name="ye1"),
                    ]
                    for e in range(2):
                        nc.tensor.matmul(
                            out=y_ps[e],
                            lhsT=outt[:, sidx * 128 : (sidx + 1) * 128],
                            rhs=wo_sb[:, e * IH : (e + 1) * IH],
                            start=True,
                            stop=True,
                        )
                    y_sb = yp.tile([128, D], F32, tag="y", name="y_sb")
                    for e in range(2):
                        nc.vector.tensor_copy(
                            out=y_sb[:, e * IH : (e + 1) * IH], in_=y_ps[e]
                        )
                    r0 = i0 + sidx * 128
                    nc.sync.dma_start(out=y_d[r0 : r0 + 128, :], in_=y_sb)

            # software pipeline: tail(c) is emitted after attn(c+1), so the
            # in-order PE queue holds next-chunk attention matmuls ahead of the
            # output projection that waits on the normalization chain.
            emit_proj(0)
            prev = None
            for c in range(nch):
                pv = emit_attn(c)
                if prev is not None:
                    emit_tail(*prev)
                if c + 1 < nch:
                    emit_proj(c + 1)
                prev = (c, pv)
            emit_tail(*prev)

            if dbg:
                nc.sync.dma_start(out=qt_dbg[:, :, :], in_=qt_sb)
                nc.sync.dma_start(out=vp_dbg[:, :, :], in_=vp_sb)
    nc.compile()
    return nc


def get_nc(T, xpart_copy=XPART_COPY, dbg=False):
    key = (T, xpart_copy, dbg)
    if key not in _NC_CACHE:
        _NC_CACHE[key] = build(T, xpart_copy, dbg)
    return _NC_CACHE[key]


TRI = np.triu(np.ones((JT, JT))).astype(NPBF)  # 1 where key j <= query i
IDENT = np.eye(128).astype(NPBF)

LAST_RESULTS = None  # BassKernelResults of the last run (for profiling)


def make_in_maps(x, Wq, Wk, Wv, Wo, n_cores=8):
    """x: (T, D) fp32. Returns per-core input maps (bf16 operands)."""
    xT = np.ascontiguousarray(x.T).astype(NPBF)
    maps = []
    for n in range(n_cores):
        sl = slice(CPC * n, CPC * (n + 1))
        maps.append(
            {
                "xT": xT,
                "wqT": np.ascontiguousarray(Wq[sl, :].T).astype(NPBF),
                "wkT": np.ascontiguousarray(Wk[sl, :].T).astype(NPBF),
                "wvT": np.ascontiguousarray(Wv[sl, :].T).astype(NPBF),
                "woT": np.ascontiguousarray(Wo[:, sl].T).astype(NPBF),
                "tri": TRI,
                "ident": IDENT,
            }
        )
    return maps


def run(x, Wq, Wk, Wv, Wo, T=None, n_cores=8, trace=False, xpart_copy=XPART_COPY,
        dbg=False):
    global LAST_RESULTS
    T = T if T is not None else x.shape[0]
    nc = get_nc(T, xpart_copy, dbg)
    in_maps = make_in_maps(x, Wq, Wk, Wv, Wo, n_cores)
    res = run_bass_kernel_spmd(
        nc, in_maps, core_ids=list(range(n_cores)), trace=trace
    )
    LAST_RESULTS = res
    y = np.zeros((T, D), dtype=np.float64)
    for r in res.results:
        y += r["y"].astype(np.float64)
    return y.astype(np.float32)


def kernel(x, Wq, Wk, Wv, Wo):
    x = np.asarray(x, dtype=np.float32)
    B, T, _ = x.shape
    trace = bool(os.environ.get("MHA_TRACE"))
    y = run(
        np.ascontiguousarray(x.reshape(T, D)),
        np.asarray(Wq, np.float32),
        np.asarray(Wk, np.float32),
        np.asarray(Wv, np.float32),
        np.asarray(Wo, np.float32),
        T=T,
        trace=trace,
    )
    if trace and LAST_RESULTS is not None and LAST_RESULTS.exec_time_ns:
        print(f"HW exec time: {LAST_RESULTS.exec_time_ns} ns")
    return y.reshape(B, T, D)



# revision 4
# speedup vs baseline: 1.2580x; 1.2580x over previous
# Multi-head causal attention (B=1, T=4096, D=1024, H=16) on 8 TRN2 NeuronCores.
#
# Sharding: tensor-parallel over heads. Core n computes head channels
# [128n, 128n+128) (= heads 2n, 2n+1), runs the full causal attention for its
# two heads, and produces a full-shape partial output
#   y_n = attn_out[:, ch_n] @ Wo[:, ch_n].T        (4096, 1024)
# The host sums the 8 partials (row-sharded Wo contraction) — no collectives.
#
# Device-side schedule (v2): the PE must never idle (HAM clock-gate throttles
# 2.4->1.2 GHz after idle windows), and the ACT engine's exp is a hard floor
# (1 elem/cycle/lane @ 1.2 GHz). So:
#  - scores for both heads land in ONE psum tile [128, h, jj, 512] (4 banks);
#    the two heads' K=64 matmuls auto-tile to PE row groups (0,0)/(64,0) and
#    run concurrently when adjacent in the queue.
#  - ONE exp per key-tile pair (2048 elem/lane) halves ACT instruction count.
#  - PV(p-1) + interleaved projection/tail work (filler thunks) keep the PE
#    busy while ACT runs exp(p).
#  - PSUM budget: 4 banks scores + 2 PV accumulators + 2 proj/tail = 8.
#
# Softmax: the ones-column appended to V (M=65) makes psum row 64 the rowsum;
# normalization via DVE reciprocal + gpsimd broadcast through DRAM scratch.

import os
import sys

for _p in ("/opt/trn_rl_repo", "/root/.axon_site/_ro/trn_rl_repo"):
    if os.path.isdir(_p) and _p not in sys.path:
        sys.path.insert(0, _p)

import ml_dtypes
import numpy as np


def _ensure_axon_ntff_hook():
    """The agent image's antenv package lacks axon_hooks, which makes
    run_bass_kernel_spmd(trace=True) crash at import under axon. Provide the
    module and register the boot hook so NTFF profiling works."""
    import types

    try:
        import antenv.axon_hooks  # noqa: F401
        return
    except ImportError:
        pass
    try:
        import antenv
    except ImportError:
        return
    mod = types.ModuleType("antenv.axon_hooks")
    mod._hook = None
    mod.set_axon_ntff_profile_hook = lambda h: setattr(mod, "_hook", h)
    mod.get_axon_ntff_profile_hook = lambda: mod._hook
    sys.modules["antenv.axon_hooks"] = mod
    antenv.axon_hooks = mod
    try:
        from trn_agent_boot.trn_boot import _ntff_profile_via_ctypes

        so = "/opt/axon/libaxon_pjrt.so"
        if os.path.exists(so):
            mod._hook = _ntff_profile_via_ctypes(so)
    except Exception:
        pass


_ensure_axon_ntff_hook()

import concourse.bass as bass  # noqa: E402
import concourse.tile as tile  # noqa: E402
from concourse import bacc  # noqa: E402
from concourse import mybir  # noqa: E402
from concourse.bass_utils import run_bass_kernel_spmd  # noqa: E402

F32 = mybir.dt.float32
BF16 = mybir.dt.bfloat16
EXP = mybir.ActivationFunctionType.Exp
NPBF = ml_dtypes.bfloat16

D = 1024          # d_model
DK = 64           # head dim
CPC = 128         # channels per core (2 heads)
ICH = 512         # query-chunk size (= psum bank free width in fp32)
JT = 128          # key-tile size

_NC_CACHE = {}


def _interleave(a, b):
    out = []
    for i in range(max(len(a), len(b))):
        if i < len(a):
            out.append(a[i])
        if i < len(b):
            out.append(b[i])
    return out


def build(T):
    """Build the per-core Bass program for sequence length T."""
    nc = bacc.Bacc(None, target_bir_lowering=False, debug=False)
    nch = T // ICH
    KD = D // 128  # contraction tiles for the projections

    xT_d = nc.dram_tensor("xT", [D, T], BF16, kind="ExternalInput")
    wqT_d = nc.dram_tensor("wqT", [D, CPC], BF16, kind="ExternalInput")
    wkT_d = nc.dram_tensor("wkT", [D, CPC], BF16, kind="ExternalInput")
    wvT_d = nc.dram_tensor("wvT", [D, CPC], BF16, kind="ExternalInput")
    woT_d = nc.dram_tensor("woT", [CPC, D], BF16, kind="ExternalInput")
    tri_d = nc.dram_tensor("tri", [JT, JT], BF16, kind="ExternalInput")
    ident_d = nc.dram_tensor("ident", [128, 128], BF16, kind="ExternalInput")
    y_d = nc.dram_tensor("y", [T, D], F32, kind="ExternalOutput")
    rs_scratch = nc.dram_tensor("rs_scratch", [nch, 2, ICH], F32)
    rs2_scratch = nc.dram_tensor("rs2_scratch", [nch, 2 * ICH], F32)

    with tile.TileContext(nc) as tc:
        with (
            tc.tile_pool(name="const", bufs=1) as const,
            tc.tile_pool(name="xtp", bufs=2) as xtp,
            tc.tile_pool(name="vtp", bufs=2) as vtp,
            tc.tile_pool(name="expp", bufs=3) as expp,
            tc.tile_pool(name="outp", bufs=2) as outp,
            tc.tile_pool(name="yp", bufs=4) as yp,
            tc.tile_pool(name="scp", bufs=1, space="PSUM") as scp,
            tc.tile_pool(name="pvp", bufs=1, space="PSUM") as pvp,
            tc.tile_pool(name="prp", bufs=1, space="PSUM") as prp,
        ):
            # ---- constants / persistent state ----
            wq_sb = const.tile([128, KD, 128], BF16)
            wk_sb = const.tile([128, KD, 128], BF16)
            wv_sb = const.tile([128, KD, 128], BF16)
            for w_sb, w_d in ((wq_sb, wqT_d), (wk_sb, wkT_d), (wv_sb, wvT_d)):
                nc.sync.dma_start(
                    out=w_sb, in_=w_d.rearrange("(t p) c -> p t c", p=128)
                )
            wo_sb = const.tile([128, D], BF16)
            nc.sync.dma_start(out=wo_sb, in_=woT_d[:, :])
            tri_sb = const.tile([JT, JT], BF16)
            nc.sync.dma_start(out=tri_sb, in_=tri_d[:, :])
            id_sb = const.tile([128, 128], BF16)
            nc.sync.dma_start(out=id_sb, in_=ident_d[:, :])

            qt_sb = const.tile([128, 2, T], BF16)  # [:,0,:]=QT, [:,1,:]=KT
            # V' = [V_h | 1] per head: [j, jt, 2*65]
            vp_sb = const.tile([128, T // JT, 2 * (DK + 1)], BF16)
            ones_view = vp_sb.rearrange("p t (h c) -> p t h c", h=2)[
                :, :, :, DK : DK + 1
            ]
            nc.vector.memset(ones_view, 1.0)

            xT_v = xT_d.rearrange("(t p) i -> p t i", p=128)
            xt_tiles = {}

            # ---------- projection thunks for chunk c ----------
            def proj_thunks(c):
                i0 = c * ICH
                box = {}

                def f_load():
                    xt = xtp.tile([128, KD, ICH], BF16, tag="xt", name="xt_ch")
                    nc.sync.dma_start(
                        out=xt, in_=xT_v[:, :, i0 : i0 + ICH]
                    )
                    xt_tiles[c] = xt

                def f_q():
                    qk_ps = prp.tile([128, 2, ICH], F32, tag="proj", name="qk_ps")
                    box["qk"] = qk_ps
                    for t in range(KD):
                        nc.tensor.matmul(
                            out=qk_ps[:, 0, :],
                            lhsT=wq_sb[:, t, :],
                            rhs=xt_tiles[c][:, t, :],
                            start=(t == 0),
                            stop=(t == KD - 1),
                        )

                def f_k():
                    qk_ps = box["qk"]
                    for t in range(KD):
                        nc.tensor.matmul(
                            out=qk_ps[:, 1, :],
                            lhsT=wk_sb[:, t, :],
                            rhs=xt_tiles[c][:, t, :],
                            start=(t == 0),
                            stop=(t == KD - 1),
                        )

                def f_qk_copy():
                    nc.vector.tensor_copy(
                        out=qt_sb[:, :, i0 : i0 + ICH], in_=box["qk"]
                    )

                def f_v():
                    vt_ps = prp.tile([128, ICH], F32, tag="proj", name="vt_ps")
                    box["vt"] = vt_ps
                    for t in range(KD):
                        nc.tensor.matmul(
                            out=vt_ps,
                            lhsT=wv_sb[:, t, :],
                            rhs=xt_tiles[c][:, t, :],
                            start=(t == 0),
                            stop=(t == KD - 1),
                        )

                def f_vt_copy():
                    vt_sb = vtp.tile([128, ICH], BF16, tag="vt", name="vt_sb")
                    box["vtsb"] = vt_sb
                    nc.vector.tensor_copy(out=vt_sb, in_=box["vt"])

                def f_transp():
                    vn_ps = prp.tile(
                        [128, ICH // 128, 128], BF16, tag="proj", name="vn_ps"
                    )
                    for sdx in range(ICH // 128):
                        nc.tensor.transpose(
                            out=vn_ps[:, sdx, :],
                            in_=box["vtsb"][:, sdx * 128 : (sdx + 1) * 128],
                            identity=id_sb,
                        )
                    jt0 = i0 // JT
                    nc.vector.tensor_copy(
                        out=vp_sb.rearrange("p t (h c) -> p t h c", h=2)[
                            :, jt0 : jt0 + ICH // 128, :, 0:DK
                        ],
                        in_=vn_ps.rearrange("p s (h c) -> p s h c", h=2),
                    )
                    del xt_tiles[c]

                return [f_load, f_q, f_k, f_qk_copy, f_v, f_vt_copy, f_transp]

            # ---------- tail thunks for chunk c ----------
            def tail_thunks(c, pv):
                i0 = c * ICH
                box = {}

                def t_rs():
                    rs = outp.tile([DK + 1, 2, ICH], F32, tag="rs", name="rs")
                    for h in range(2):
                        nc.vector.tensor_copy(
                            out=rs[DK : DK + 1, h, :], in_=pv[h][DK : DK + 1, :]
                        )
                    nc.sync.dma_start(
                        out=rs_scratch[c : c + 1, :, :], in_=rs[DK : DK + 1, :, :]
                    )

                def t_outt():
                    outt = outp.tile([128, ICH], BF16, tag="outt", name="outt")
                    box["outt"] = outt
                    for h in range(2):
                        nc.vector.tensor_copy(
                            out=outt[h * DK : (h + 1) * DK, :],
                            in_=pv[h][0:DK, :],
                        )

                def t_recip():
                    rsp = outp.tile(
                        [128, 2 * ICH // 128], F32, tag="rsp", name="rsp"
                    )
                    nc.sync.dma_start(
                        out=rsp,
                        in_=rs_scratch[c]
                        .rearrange("h i -> (h i)")
                        .rearrange("(p f) -> p f", p=128),
                    )
                    nc.vector.reciprocal(out=rsp, in_=rsp)
                    nc.sync.dma_start(
                        out=rs2_scratch[c].rearrange("(p f) -> p f", p=128),
                        in_=rsp,
                    )

                def t_bc():
                    bc = outp.tile([128, ICH], F32, tag="bc", name="bc")
                    box["bc"] = bc
                    for h in range(2):
                        nc.gpsimd.dma_start(
                            out=bc[h * DK : (h + 1) * DK, :],
                            in_=rs2_scratch[c]
                            .rearrange("(h i) -> h i", h=2)[h : h + 1, :]
                            .to_broadcast([DK, ICH]),
                        )

                def t_mul():
                    nc.vector.tensor_mul(box["outt"], box["outt"], box["bc"])

                def mk_y(sidx):
                    def t_y():
                        y_ps = prp.tile(
                            [128, 2, ICH], F32, tag="proj", name="y_ps"
                        )
                        for e in range(2):
                            nc.tensor.matmul(
                                out=y_ps[:, e, :],
                                lhsT=box["outt"][
                                    :, sidx * 128 : (sidx + 1) * 128
                                ],
                                rhs=wo_sb[:, e * ICH : (e + 1) * ICH],
                                start=True,
                                stop=True,
                            )
                        y_sb = yp.tile([128, D], F32, tag="y", name="y_sb")
                        nc.vector.tensor_copy(
                            out=y_sb.rearrange("p (e i) -> p e i", e=2),
                            in_=y_ps,
                        )
                        r0 = i0 + sidx * 128
                        nc.sync.dma_start(out=y_d[r0 : r0 + 128, :], in_=y_sb)

                    return t_y

                return [t_rs, t_outt, t_recip, t_bc, t_mul] + [
                    mk_y(s) for s in range(ICH // 128)
                ]

            # ---------- main schedule ----------
            filler = []

            def pop_filler(n=1):
                for _ in range(n):
                    if filler:
                        filler.pop(0)()

            def emit_pv(pv, p, ex, njt):
                for h in range(2):
                    for jj in range(2):
                        jt = 2 * p + jj
                        nc.tensor.matmul(
                            out=pv[h][0 : DK + 1, :],
                            lhsT=vp_sb[
                                :, jt, h * (DK + 1) : (h + 1) * (DK + 1)
                            ],
                            rhs=ex[:, h, jj, :],
                            start=(jt == 0),
                            stop=(jt == njt - 1),
                        )

            for f in proj_thunks(0):
                f()

            prev = None
            for c in range(nch):
                njt = (c + 1) * (ICH // JT)
                i0 = c * ICH
                pv = [
                    pvp.tile([128, ICH], F32, tag=f"pv{h}", name=f"pv{h}")
                    for h in range(2)
                ]
                tl = tail_thunks(*prev) if prev is not None else []
                pj = proj_thunks(c + 1) if c + 1 < nch else []
                filler = _interleave(tl, pj)

                prev_pair = None
                for p in range(njt // 2):
                    sc = scp.tile([128, 2, 2, ICH], F32, tag="sc", name="sc")
                    for jj in range(2):
                        jt = 2 * p + jj
                        for h in range(2):
                            hp = slice(h * DK, (h + 1) * DK)
                            nc.tensor.matmul(
                                out=sc[:, h, jj, :],
                                lhsT=qt_sb[hp, 1, jt * JT : (jt + 1) * JT],
                                rhs=qt_sb[hp, 0, i0 : i0 + ICH],
                                start=True,
                                stop=True,
                            )
                    pop_filler(1)
                    if prev_pair is not None:
                        emit_pv(pv, *prev_pair)
                        pop_filler(1)
                    ex = expp.tile([128, 2, 2, ICH], BF16, tag="ex", name="ex")
                    nc.scalar.activation(
                        out=ex, in_=sc, func=EXP, scale=1.0 / np.sqrt(DK)
                    )
                    # causal mask on diagonal-straddling tiles
                    for jj in range(2):
                        jt = 2 * p + jj
                        off = jt * JT - i0
                        if off >= 0:
                            for h in range(2):
                                if off > 0:
                                    nc.vector.memset(ex[:, h, jj, 0:off], 0.0)
                                nc.vector.tensor_mul(
                                    ex[:, h, jj, off : off + JT],
                                    ex[:, h, jj, off : off + JT],
                                    tri_sb,
                                )
                    prev_pair = (p, ex, njt)
                emit_pv(pv, *prev_pair)
                while filler:
                    pop_filler(1)
                prev = (c, pv)
            for f in tail_thunks(*prev):
                f()

    nc.compile()
    return nc


def get_nc(T):
    if T not in _NC_CACHE:
        _NC_CACHE[T] = build(T)
    return _NC_CACHE[T]


TRI = np.triu(np.ones((JT, JT))).astype(NPBF)  # 1 where key j <= query i
IDENT = np.eye(128).astype(NPBF)

LAST_RESULTS = None  # BassKernelResults of the last run (for profiling)


def make_in_maps(x, Wq, Wk, Wv, Wo, n_cores=8):
    """x: (T, D) fp32. Returns per-core input maps (bf16 operands)."""
    xT = np.ascontiguousarray(x.T).astype(NPBF)
    maps = []
    for n in range(n_cores):
        sl = slice(CPC * n, CPC * (n + 1))
        maps.append(
            {
                "xT": xT,
                "wqT": np.ascontiguousarray(Wq[sl, :].T).astype(NPBF),
                "wkT": np.ascontiguousarray(Wk[sl, :].T).astype(NPBF),
                "wvT": np.ascontiguousarray(Wv[sl, :].T).astype(NPBF),
                "woT": np.ascontiguousarray(Wo[:, sl].T).astype(NPBF),
                "tri": TRI,
                "ident": IDENT,
            }
        )
    return maps


def run(x, Wq, Wk, Wv, Wo, T=None, n_cores=8, trace=False):
    global LAST_RESULTS
    T = T if T is not None else x.shape[0]
    nc = get_nc(T)
    in_maps = make_in_maps(x, Wq, Wk, Wv, Wo, n_cores)
    res = run_bass_kernel_spmd(
        nc, in_maps, core_ids=list(range(n_cores)), trace=trace
    )
    LAST_RESULTS = res
    y = np.zeros((T, D), dtype=np.float64)
    for r in res.results:
        y += r["y"].astype(np.float64)
    return y.astype(np.float32)


def kernel(x, Wq, Wk, Wv, Wo):
    x = np.asarray(x, dtype=np.float32)
    B, T, _ = x.shape
    trace = bool(os.environ.get("MHA_TRACE"))
    y = run(
        np.ascontiguousarray(x.reshape(T, D)),
        np.asarray(Wq, np.float32),
        np.asarray(Wk, np.float32),
        np.asarray(Wv, np.float32),
        np.asarray(Wo, np.float32),
        T=T,
        trace=trace,
    )
    if trace and LAST_RESULTS is not None and LAST_RESULTS.exec_time_ns:
        print(f"HW exec time: {LAST_RESULTS.exec_time_ns} ns")
    return y.reshape(B, T, D)
